# revision 26
# baseline (speedup 1.0000x reference)
"""Trainium2 Bass kernel for the DTI predictor (gnn_message_passing).

Math (reference):
  a_mol = mol_feats @ Wmu[:H] + bmu            [N, heads]
  a_pro = fused_feats @ Wmu[H:]                [P, heads]
  y_atom[n,h] = sum_p ( elu(a_mol[n,h] + a_pro[p,h]) + 1 )
  y = segment_sum(y_atom, mol_batch, B) * 1e-3
  out = elu(y @ W1 + b1) @ W2 + b2             [B, 1]

Key identity:   elu(x) + 1 = relu(x) + min(exp(a_mol)*exp(a_pro), 1)

The exp ("s") part runs in p-on-partition layout: one 4x-mode DVE
dual-op per [128p, N] tile (s = (bcast(exp(a_mol)) * exp(a_pro)[p]) min 1)
and the TensorEngine reduces over p via ones-matmuls accumulating into a
[1, N] PSUM row. The relu ("r") part is split: atom tiles t < A2T run
fused on the Scalar engine (relu(bcast(a_pro) + a_mol[n]) with
accum_out, n-on-partition layout); the remaining atom range runs like
the s part on DVE+PE, with the covered range poisoned to -1e9 in the
broadcast row so relu contributes exactly zero there (no double count).

Sharding: 16 heads across 8 cores (2 each, full N and P). Core output:
"out" [B, 2] (device-pooled ACT-range contributions, already *1e-3) and
"yrow" [2, N] (per-atom row sums from the DVE/PE path); host adds the
segment-sum of yrow, concats head slices, and applies the tiny MLP.
"""

import sys

sys.path.insert(0, "/opt/trn_rl_repo")

import numpy as np
import ml_dtypes

import concourse.bass as bass
import concourse.tile as tile
import concourse.bacc as bacc
from concourse import mybir
from concourse.bass_utils import run_bass_kernel_spmd

N_MOL, P_PRO, HID, HEADS, B = 2048, 2048, 64, 16, 64
N_CORES = 8
HPC = HEADS // N_CORES          # heads per core = 2
NT = N_MOL // 128               # atom partition-tiles = 16
NQ = P_PRO // 128               # protein partition-tiles = 16
NCH = P_PRO // 512              # 512-col chunks = 4
F32 = mybir.dt.float32
BF16 = mybir.dt.bfloat16
I32 = mybir.dt.int32
ALU = mybir.AluOpType
AF = mybir.ActivationFunctionType

A2T = 12                        # atom-tiles per head with relu on ACT
RSTART = 1536                   # p-layout r covers n in [RSTART, N); must be
                                # 512-aligned and <= A2T*128 (poison covers rest)
RW = N_MOL - RSTART             # r dual width
RCH0 = RSTART // 512


def build():
    assert RSTART <= A2T * 128 and RSTART % 512 == 0
    nc = bacc.Bacc("TRN2", target_bir_lowering=False, debug=False,
                   num_devices=N_CORES)
    molT_d = nc.dram_tensor("molT", [HID + 1, N_MOL], BF16, kind="ExternalInput").ap()
    fusedT_d = nc.dram_tensor("fusedT", [HID, P_PRO], BF16, kind="ExternalInput").ap()
    wmol_d = nc.dram_tensor("wmol", [HID + 1, HPC], BF16, kind="ExternalInput").ap()
    wpro_d = nc.dram_tensor("wpro", [HID, HPC], BF16, kind="ExternalInput").ap()
    masks_d = nc.dram_tensor("masks", [128, A2T * B], BF16, kind="ExternalInput").ap()
    out_d = nc.dram_tensor("out", [B, HPC], F32, kind="ExternalOutput").ap()
    yrow_d = nc.dram_tensor("yrow", [HPC, N_MOL], F32, kind="ExternalOutput").ap()

    with tile.TileContext(nc) as tc:
        with (
            tc.tile_pool(name="const", bufs=1) as cpool,
            tc.tile_pool(name="bc", bufs=2) as bcpool,
            tc.tile_pool(name="cols", bufs=NT) as colpool,
            tc.tile_pool(name="rows", bufs=1) as rowpool,
            tc.tile_pool(name="work", bufs=4) as wpool,
            tc.tile_pool(name="junk", bufs=2) as jpool,
            tc.tile_pool(name="small", bufs=4) as spool,
            tc.tile_pool(name="ps", bufs=1, space=bass.MemorySpace.PSUM) as pspool,
            tc.tile_pool(name="pssm", bufs=2, space=bass.MemorySpace.PSUM) as smpool,
            tc.tile_pool(name="psrow", bufs=1, space=bass.MemorySpace.PSUM) as rwpool,
            tc.tile_pool(name="psacc", bufs=1, space=bass.MemorySpace.PSUM) as accpool,
        ):
            # ---- inputs ----
            molT = cpool.tile([HID + 1, N_MOL], BF16, tag="molT")
            fusedT = cpool.tile([HID, P_PRO], BF16, tag="fusedT")
            wmol = cpool.tile([HID + 1, HPC], BF16, tag="wmol")
            wpro = cpool.tile([HID, HPC], BF16, tag="wpro")
            masks = cpool.tile([128, A2T * B], BF16, tag="masks")
            # molT gates the longest chain (a_mol -> emrow -> bc_e); put it
            # first and spread big loads across separate DMA queues.
            nc.scalar.dma_start(wmol[:], wmol_d)
            nc.scalar.dma_start(wpro[:], wpro_d)
            for j in range(NCH):
                nc.sync.dma_start(molT[:, bass.ts(j, 512)], molT_d[:, bass.ts(j, 512)])
            for j in range(NCH):
                nc.gpsimd.dma_start(fusedT[:, bass.ts(j, 512)], fusedT_d[:, bass.ts(j, 512)])
            nc.scalar.dma_start(masks[:], masks_d)

            # ---- constants: ones column, head-select tiles, f32 identity ----
            ones = cpool.tile([128, 1], BF16, tag="ones")
            nc.vector.memset(ones[:], 1.0)
            # sel[h]: [HPC, 128] with row h all-ones -> ones-matmul broadcasts
            # row h of a [HPC, N] row-pair without slicing its partition base.
            iota_p2 = cpool.tile([HPC, 128], F32, tag="iota_p2")
            nc.gpsimd.iota(iota_p2[:], pattern=[[0, 128]], base=0,
                           channel_multiplier=1,
                           allow_small_or_imprecise_dtypes=True)
            sel = []
            for h in range(HPC):
                s = cpool.tile([HPC, 128], BF16, tag=f"sel{h}", name=f"sel{h}")
                nc.vector.tensor_scalar(s[:], iota_p2[:], float(h), None,
                                        ALU.is_equal, ALU.bypass)
                sel.append(s)
            iota_f = cpool.tile([128, 128], F32, tag="iota_f")
            nc.gpsimd.iota(iota_f[:], pattern=[[1, 128]], base=0, channel_multiplier=0,
                           allow_small_or_imprecise_dtypes=True)
            pidx = cpool.tile([128, 1], F32, tag="pidx")
            nc.gpsimd.iota(pidx[:], pattern=[[1, 1]], base=0, channel_multiplier=1,
                           allow_small_or_imprecise_dtypes=True)
            ident = cpool.tile([128, 128], F32, tag="ident")
            nc.vector.tensor_scalar(ident[:], iota_f[:], pidx[:], None,
                                    ALU.is_equal, ALU.bypass)

            def build_bc(dst, src_rows, h, col0, ncols):
                """dst[:, :] = broadcast of src_rows[h, col0:col0+ncols]."""
                for j in range(ncols // 512):
                    bc_ps = pspool.tile([128, 512], F32, tag="bc_ps")
                    nc.tensor.matmul(bc_ps[:], sel[h][:],
                                     src_rows[:, col0 + j * 512:col0 + (j + 1) * 512],
                                     start=True, stop=True)
                    nc.vector.tensor_copy(dst[:, bass.ts(j, 512)], bc_ps[:])

            bc_a, bc_m, bc_e = [], [], []
            for h in range(HPC):
                bc_a.append(bcpool.tile([128, P_PRO], BF16, tag="bca", name=f"bca{h}"))
                bc_m.append(bcpool.tile([128, RW], BF16, tag="bcm", name=f"bcm{h}"))
                bc_e.append(bcpool.tile([128, N_MOL], BF16, tag="bce", name=f"bce{h}"))

            # ---- a_mol chain first: it gates the DVE main loop via bc_e ----
            am = []
            row_ps2 = rwpool.tile([HPC, N_MOL], F32, tag="bigrow", name="row_ps_am")
            for t in range(NT):
                am_ps = smpool.tile([128, HPC], F32, tag="am_ps")
                nc.tensor.matmul(am_ps[:], molT[:, bass.ts(t, 128)], wmol[:],
                                 start=True, stop=True)
                amt = colpool.tile([128, HPC], F32, tag="am", name=f"am{t}")
                nc.vector.tensor_copy(amt[:], am_ps[:])
                am.append(amt)
            for t in range(NT):
                nc.tensor.transpose(row_ps2[:, bass.ts(t, 128)], am[t][:], ident[:])
            amrow = rowpool.tile([HPC, N_MOL], BF16, tag="amrow")
            emrow = rowpool.tile([HPC, N_MOL], BF16, tag="emrow")
            nc.scalar.activation(emrow[:], row_ps2[:], AF.Exp)
            nc.vector.tensor_copy(amrow[:], row_ps2[:])
            # poison the ACT-covered atom range so p-layout relu contributes 0
            nc.vector.memset(amrow[:, 0:A2T * 128], -1e9)
            for h in range(HPC):
                build_bc(bc_e[h], emrow, h, 0, N_MOL)
                build_bc(bc_m[h], amrow, h, RSTART, RW)

            # ---- a_pro columns + exp; then its row / bc_a (gates only ACT) ----
            apc, epc = [], []
            row_ps = rwpool.tile([HPC, N_MOL], F32, tag="bigrow", name="row_ps_ap")
            for q in range(NQ):
                ap_ps = smpool.tile([128, HPC], F32, tag="am_ps")
                nc.tensor.matmul(ap_ps[:], fusedT[:, bass.ts(q, 128)], wpro[:],
                                 start=True, stop=True)
                a = colpool.tile([128, HPC], F32, tag="apc", name=f"apc{q}")
                nc.vector.tensor_copy(a[:], ap_ps[:])
                e = colpool.tile([128, HPC], F32, tag="epc", name=f"epc{q}")
                nc.scalar.activation(e[:], ap_ps[:], AF.Exp)
                apc.append(a)
                epc.append(e)
            for q in range(NQ):
                nc.tensor.transpose(row_ps[:, bass.ts(q, 128)], apc[q][:], ident[:])
            aprow = rowpool.tile([HPC, P_PRO], BF16, tag="aprow")
            nc.vector.tensor_copy(aprow[:], row_ps[:])
            for h in range(HPC):
                build_bc(bc_a[h], aprow, h, 0, P_PRO)

            # ---- pooled accumulator for ACT-range columns ----
            pool_ps = accpool.tile([B, HPC], F32, tag="pool_ps")
            n_pool = A2T * HPC
            pool_i = 0

            # ---- main loops: heads sequential (one PSUM row accumulator) ----
            for h in range(HPC):
                yrow_ps = rwpool.tile([1, N_MOL], F32, tag="bigrow", name=f"yrow{h}")
                st_hold = None
                for q in range(NQ):
                    st = wpool.tile([128, N_MOL], BF16, tag="st")
                    nc.vector.tensor_scalar(st[:], bc_e[h][:], epc[q][:, h:h + 1],
                                            1.0, ALU.mult, ALU.min)
                    # pair-fold the first 8 q-tiles on DVE: one tensor_add
                    # replaces four PE reduce matmuls
                    mm_src = None
                    if q < 8:
                        if q % 2 == 0:
                            st_hold = st
                        else:
                            stsum = wpool.tile([128, N_MOL], BF16, tag="stsum")
                            nc.vector.tensor_add(stsum[:], st_hold[:], st[:])
                            mm_src = stsum
                            s_start = (q == 1)
                    else:
                        mm_src = st
                        s_start = False
                    if mm_src is not None:
                        for c in range(NCH):
                            nc.tensor.matmul(
                                yrow_ps[:, bass.ts(c, 512)], ones[:],
                                mm_src[:, bass.ts(c, 512)],
                                start=(s_start and c < RCH0),
                                stop=(q == NQ - 1 and c < RCH0))
                    rt = wpool.tile([128, RW], BF16, tag="rt")
                    nc.vector.tensor_scalar(rt[:], bc_m[h][:], apc[q][:, h:h + 1],
                                            0.0, ALU.add, ALU.max)
                    for c in range(RCH0, NCH):
                        nc.tensor.matmul(yrow_ps[:, bass.ts(c, 512)], ones[:],
                                         rt[:, bass.ts(c - RCH0, 512)],
                                         start=(q == 0), stop=(q == NQ - 1))
                    # interleave ACT relu (n-layout, fused accum) so the
                    # Scalar engine starts early instead of queuing at the end
                    if q < A2T:
                        t = q
                        rjunk = jpool.tile([128, P_PRO], BF16, tag="rjunk")
                        racc = spool.tile([128, 1], F32, tag="racc")
                        nc.scalar.activation(rjunk[:], bc_a[h][:], AF.Relu,
                                             bias=am[t][:, h:h + 1],
                                             accum_out=racc[:])
                        rb = spool.tile([128, 1], BF16, tag="rb")
                        nc.vector.tensor_copy(rb[:], racc[:])
                        nc.tensor.matmul(pool_ps[:, h:h + 1],
                                         masks[:, bass.ts(t, B)],
                                         rb[:], start=(pool_i == 0),
                                         stop=(pool_i == n_pool - 1))
                        pool_i += 1
                # drain the row accumulator to SBUF then DRAM
                yrow_sb = spool.tile([1, N_MOL], F32, tag="yrow_sb")
                nc.vector.tensor_copy(yrow_sb[:], yrow_ps[:])
                nc.sync.dma_start(yrow_d[h:h + 1, :], yrow_sb[:])

            out_sb = spool.tile([B, HPC], F32, tag="out_sb")
            nc.scalar.activation(out_sb[:], pool_ps[:], AF.Copy, scale=0.001)
            nc.sync.dma_start(out_d, out_sb[:])

    nc.compile()
    return nc


_NC = None


def _get_nc():
    global _NC
    if _NC is None:
        _NC = build()
    return _NC


def make_in_maps(mol_feats, fused_feats, Wmu, bmu, mol_batch):
    """Host-side sharding: per-core input dicts."""
    bf = ml_dtypes.bfloat16
    molT = np.concatenate([np.asarray(mol_feats, np.float32).T,
                           np.ones((1, N_MOL), np.float32)], axis=0)
    molT = np.ascontiguousarray(molT).astype(bf)
    fusedT = np.ascontiguousarray(np.asarray(fused_feats, np.float32).T).astype(bf)
    Wmu = np.asarray(Wmu, np.float32)
    bmu = np.asarray(bmu, np.float32)
    mb = np.asarray(mol_batch).astype(np.int64)
    masks = np.zeros((128, A2T * B), np.float32)
    for t in range(A2T):
        seg = mb[t * 128:(t + 1) * 128]
        masks[np.arange(128), t * B + seg] = 1.0
    masks = masks.astype(bf)

    in_maps = []
    for c in range(N_CORES):
        h0 = c * HPC
        wmol = np.ascontiguousarray(
            np.concatenate([Wmu[:HID, h0:h0 + HPC], bmu[None, h0:h0 + HPC]],
                           axis=0)).astype(bf)
        wpro = np.ascontiguousarray(Wmu[HID:, h0:h0 + HPC]).astype(bf)
        in_maps.append({
            "molT": molT, "fusedT": fusedT,
            "wmol": wmol, "wpro": wpro, "masks": masks,
        })
    return in_maps


def _elu(v):
    return np.where(v > 0, v, np.expm1(v))


def combine(results, mol_batch):
    """Per-core outputs -> pooled [B, HEADS] f32 (already * 1e-3)."""
    mb = np.asarray(mol_batch).astype(np.int64)
    pooled = np.zeros((B, HEADS), np.float32)
    for c in range(N_CORES):
        h0 = c * HPC
        pooled[:, h0:h0 + HPC] += results[c]["out"]
        yrow = results[c]["yrow"]          # [HPC, N] f32
        for h in range(HPC):
            pooled[:, h0 + h] += 1e-3 * np.bincount(
                mb, weights=yrow[h].astype(np.float64), minlength=B
            ).astype(np.float32)
    return pooled


def finish(pooled, W1, b1, W2, b2):
    y = _elu(pooled @ np.asarray(W1, np.float32) + np.asarray(b1, np.float32))
    return (y @ np.asarray(W2, np.float32) + np.asarray(b2, np.float32)).astype(np.float32)


def kernel(mol_feats, fused_feats, Wmu, bmu, W1, b1, W2, b2, mol_batch,
           num_graphs, **_unused):
    nc = _get_nc()
    in_maps = make_in_maps(mol_feats, fused_feats, Wmu, bmu, mol_batch)
    res = run_bass_kernel_spmd(nc, in_maps, core_ids=list(range(N_CORES)))
    pooled = combine(res.results, mol_batch)
    return finish(pooled, W1, b1, W2, b2)


# revision 27
# speedup vs baseline: 1.0173x; 1.0173x over previous
"""Trainium2 Bass kernel for the DTI predictor (gnn_message_passing).

Math (reference):
  a_mol = mol_feats @ Wmu[:H] + bmu            [N, heads]
  a_pro = fused_feats @ Wmu[H:]                [P, heads]
  y_atom[n,h] = sum_p ( elu(a_mol[n,h] + a_pro[p,h]) + 1 )
  y = segment_sum(y_atom, mol_batch, B) * 1e-3
  out = elu(y @ W1 + b1) @ W2 + b2             [B, 1]

Key identity:   elu(x) + 1 = relu(x) + min(exp(a_mol)*exp(a_pro), 1)

The exp ("s") part runs in p-on-partition layout: one 4x-mode DVE
dual-op per [128p, N] tile (s = (bcast(exp(a_mol)) * exp(a_pro)[p]) min 1)
and the TensorEngine reduces over p via ones-matmuls accumulating into a
[1, N] PSUM row. The relu ("r") part is split: atom tiles t < A2T run
fused on the Scalar engine (relu(bcast(a_pro) + a_mol[n]) with
accum_out, n-on-partition layout); the remaining atom range runs like
the s part on DVE+PE, with the covered range poisoned to -1e9 in the
broadcast row so relu contributes exactly zero there (no double count).

Sharding: 16 heads across 8 cores (2 each, full N and P). Core output:
"out" [B, 2] (device-pooled ACT-range contributions, already *1e-3) and
"yrow" [2, N] (per-atom row sums from the DVE/PE path); host adds the
segment-sum of yrow, concats head slices, and applies the tiny MLP.
"""

import sys

sys.path.insert(0, "/opt/trn_rl_repo")

import numpy as np
import ml_dtypes

import concourse.bass as bass
import concourse.tile as tile
import concourse.bacc as bacc
from concourse import mybir
from concourse.bass_utils import run_bass_kernel_spmd

N_MOL, P_PRO, HID, HEADS, B = 2048, 2048, 64, 16, 64
N_CORES = 8
HPC = HEADS // N_CORES          # heads per core = 2
NT = N_MOL // 128               # atom partition-tiles = 16
NQ = P_PRO // 128               # protein partition-tiles = 16
NCH = P_PRO // 512              # 512-col chunks = 4
F32 = mybir.dt.float32
BF16 = mybir.dt.bfloat16
I32 = mybir.dt.int32
ALU = mybir.AluOpType
AF = mybir.ActivationFunctionType

A2T = 12                        # atom-tiles per head with relu on ACT
RSTART = 1536                   # p-layout r covers n in [RSTART, N); must be
                                # 512-aligned and <= A2T*128 (poison covers rest)
RW = N_MOL - RSTART             # r dual width
RCH0 = RSTART // 512


def build():
    assert RSTART <= A2T * 128 and RSTART % 512 == 0
    nc = bacc.Bacc("TRN2", target_bir_lowering=False, debug=False,
                   num_devices=N_CORES)
    molT_d = nc.dram_tensor("molT", [HID + 1, N_MOL], BF16, kind="ExternalInput").ap()
    fusedT_d = nc.dram_tensor("fusedT", [HID, P_PRO], BF16, kind="ExternalInput").ap()
    wmol_d = nc.dram_tensor("wmol", [HID + 1, HPC], BF16, kind="ExternalInput").ap()
    wpro_d = nc.dram_tensor("wpro", [HID, HPC], BF16, kind="ExternalInput").ap()
    masks_d = nc.dram_tensor("masks", [128, A2T * B], BF16, kind="ExternalInput").ap()
    out_d = nc.dram_tensor("out", [B, HPC], F32, kind="ExternalOutput").ap()
    yrow_d = nc.dram_tensor("yrow", [HPC, N_MOL], F32, kind="ExternalOutput").ap()

    with tile.TileContext(nc) as tc:
        with (
            tc.tile_pool(name="const", bufs=1) as cpool,
            tc.tile_pool(name="bc", bufs=2) as bcpool,
            tc.tile_pool(name="cols", bufs=NT) as colpool,
            tc.tile_pool(name="rows", bufs=1) as rowpool,
            tc.tile_pool(name="work", bufs=4) as wpool,
            tc.tile_pool(name="junk", bufs=2) as jpool,
            tc.tile_pool(name="small", bufs=4) as spool,
            tc.tile_pool(name="ps", bufs=1, space=bass.MemorySpace.PSUM) as pspool,
            tc.tile_pool(name="pssm", bufs=2, space=bass.MemorySpace.PSUM) as smpool,
            tc.tile_pool(name="psrow", bufs=1, space=bass.MemorySpace.PSUM) as rwpool,
            tc.tile_pool(name="psacc", bufs=1, space=bass.MemorySpace.PSUM) as accpool,
        ):
            # ---- inputs ----
            molT = cpool.tile([HID + 1, N_MOL], BF16, tag="molT")
            fusedT = cpool.tile([HID, P_PRO], BF16, tag="fusedT")
            wmol = cpool.tile([HID + 1, HPC], BF16, tag="wmol")
            wpro = cpool.tile([HID, HPC], BF16, tag="wpro")
            masks = cpool.tile([128, A2T * B], BF16, tag="masks")
            # molT gates the longest chain (a_mol -> emrow -> bc_e); put it
            # first and spread big loads across separate DMA queues.
            nc.scalar.dma_start(wmol[:], wmol_d)
            nc.scalar.dma_start(wpro[:], wpro_d)
            for j in range(NCH):
                nc.sync.dma_start(molT[:, bass.ts(j, 512)], molT_d[:, bass.ts(j, 512)])
            for j in range(NCH):
                nc.gpsimd.dma_start(fusedT[:, bass.ts(j, 512)], fusedT_d[:, bass.ts(j, 512)])
            nc.scalar.dma_start(masks[:], masks_d)

            # ---- constants: ones column, head-select tiles, f32 identity ----
            ones = cpool.tile([128, 1], BF16, tag="ones")
            nc.vector.memset(ones[:], 1.0)
            # sel[h]: [HPC, 128] with row h all-ones -> ones-matmul broadcasts
            # row h of a [HPC, N] row-pair without slicing its partition base.
            iota_p2 = cpool.tile([HPC, 128], F32, tag="iota_p2")
            nc.gpsimd.iota(iota_p2[:], pattern=[[0, 128]], base=0,
                           channel_multiplier=1,
                           allow_small_or_imprecise_dtypes=True)
            sel = []
            for h in range(HPC):
                s = cpool.tile([HPC, 128], BF16, tag=f"sel{h}", name=f"sel{h}")
                nc.vector.tensor_scalar(s[:], iota_p2[:], float(h), None,
                                        ALU.is_equal, ALU.bypass)
                sel.append(s)
            iota_f = cpool.tile([128, 128], F32, tag="iota_f")
            nc.gpsimd.iota(iota_f[:], pattern=[[1, 128]], base=0, channel_multiplier=0,
                           allow_small_or_imprecise_dtypes=True)
            pidx = cpool.tile([128, 1], F32, tag="pidx")
            nc.gpsimd.iota(pidx[:], pattern=[[1, 1]], base=0, channel_multiplier=1,
                           allow_small_or_imprecise_dtypes=True)
            ident = cpool.tile([128, 128], F32, tag="ident")
            nc.vector.tensor_scalar(ident[:], iota_f[:], pidx[:], None,
                                    ALU.is_equal, ALU.bypass)

            def build_bc(dst, src_rows, h, col0, ncols):
                """dst[:, :] = broadcast of src_rows[h, col0:col0+ncols]."""
                for j in range(ncols // 512):
                    bc_ps = pspool.tile([128, 512], F32, tag="bc_ps")
                    nc.tensor.matmul(bc_ps[:], sel[h][:],
                                     src_rows[:, col0 + j * 512:col0 + (j + 1) * 512],
                                     start=True, stop=True)
                    nc.vector.tensor_copy(dst[:, bass.ts(j, 512)], bc_ps[:])

            bc_a, bc_m, bc_e = [], [], []
            for h in range(HPC):
                bc_a.append(bcpool.tile([128, P_PRO], BF16, tag="bca", name=f"bca{h}"))
                bc_m.append(bcpool.tile([128, RW], BF16, tag="bcm", name=f"bcm{h}"))
                bc_e.append(bcpool.tile([128, N_MOL], BF16, tag="bce", name=f"bce{h}"))

            # ---- a_mol chain first: it gates the DVE main loop via bc_e ----
            am = []
            row_ps2 = rwpool.tile([HPC, N_MOL], F32, tag="bigrow", name="row_ps_am")
            for t in range(NT):
                am_ps = smpool.tile([128, HPC], F32, tag="am_ps")
                nc.tensor.matmul(am_ps[:], molT[:, bass.ts(t, 128)], wmol[:],
                                 start=True, stop=True)
                amt = colpool.tile([128, HPC], F32, tag="am", name=f"am{t}")
                nc.vector.tensor_copy(amt[:], am_ps[:])
                am.append(amt)
            for t in range(NT):
                nc.tensor.transpose(row_ps2[:, bass.ts(t, 128)], am[t][:], ident[:])
            amrow = rowpool.tile([HPC, N_MOL], BF16, tag="amrow")
            emrow = rowpool.tile([HPC, N_MOL], BF16, tag="emrow")
            nc.scalar.activation(emrow[:], row_ps2[:], AF.Exp)
            nc.vector.tensor_copy(amrow[:], row_ps2[:])
            # poison the ACT-covered atom range so p-layout relu contributes 0
            nc.vector.memset(amrow[:, 0:A2T * 128], -1e9)
            for h in range(HPC):
                build_bc(bc_e[h], emrow, h, 0, N_MOL)
                build_bc(bc_m[h], amrow, h, RSTART, RW)

            # ---- a_pro columns + exp; then its row / bc_a (gates only ACT) ----
            apc, epc = [], []
            row_ps = rwpool.tile([HPC, N_MOL], F32, tag="bigrow", name="row_ps_ap")
            for q in range(NQ):
                ap_ps = smpool.tile([128, HPC], F32, tag="am_ps")
                nc.tensor.matmul(ap_ps[:], fusedT[:, bass.ts(q, 128)], wpro[:],
                                 start=True, stop=True)
                a = colpool.tile([128, HPC], F32, tag="apc", name=f"apc{q}")
                nc.vector.tensor_copy(a[:], ap_ps[:])
                e = colpool.tile([128, HPC], F32, tag="epc", name=f"epc{q}")
                nc.scalar.activation(e[:], ap_ps[:], AF.Exp)
                apc.append(a)
                epc.append(e)
            for q in range(NQ):
                nc.tensor.transpose(row_ps[:, bass.ts(q, 128)], apc[q][:], ident[:])
            aprow = rowpool.tile([HPC, P_PRO], BF16, tag="aprow")
            nc.vector.tensor_copy(aprow[:], row_ps[:])
            for h in range(HPC):
                build_bc(bc_a[h], aprow, h, 0, P_PRO)

            # ---- pooled accumulator for ACT-range columns ----
            pool_ps = accpool.tile([B, HPC], F32, tag="pool_ps")
            n_pool = A2T * HPC
            pool_i = 0

            # ---- main loops: heads sequential (one PSUM row accumulator) ----
            for h in range(HPC):
                yrow_ps = rwpool.tile([1, N_MOL], F32, tag="bigrow", name=f"yrow{h}")
                for q in range(NQ):
                    st = wpool.tile([128, N_MOL], BF16, tag="st")
                    nc.vector.tensor_scalar(st[:], bc_e[h][:], epc[q][:, h:h + 1],
                                            1.0, ALU.mult, ALU.min)
                    for c in range(NCH):
                        nc.tensor.matmul(yrow_ps[:, bass.ts(c, 512)], ones[:],
                                         st[:, bass.ts(c, 512)],
                                         start=(q == 0),
                                         stop=(q == NQ - 1 and c < RCH0))
                    rt = wpool.tile([128, RW], BF16, tag="rt")
                    nc.vector.tensor_scalar(rt[:], bc_m[h][:], apc[q][:, h:h + 1],
                                            0.0, ALU.add, ALU.max)
                    for c in range(RCH0, NCH):
                        nc.tensor.matmul(yrow_ps[:, bass.ts(c, 512)], ones[:],
                                         rt[:, bass.ts(c - RCH0, 512)],
                                         start=False, stop=(q == NQ - 1))
                    # interleave ACT relu (n-layout, fused accum) so the
                    # Scalar engine starts early instead of queuing at the end
                    if q < A2T:
                        t = q
                        rjunk = jpool.tile([128, P_PRO], BF16, tag="rjunk")
                        racc = spool.tile([128, 1], F32, tag="racc")
                        nc.scalar.activation(rjunk[:], bc_a[h][:], AF.Relu,
                                             bias=am[t][:, h:h + 1],
                                             accum_out=racc[:])
                        rb = spool.tile([128, 1], BF16, tag="rb")
                        nc.vector.tensor_copy(rb[:], racc[:])
                        nc.tensor.matmul(pool_ps[:, h:h + 1],
                                         masks[:, bass.ts(t, B)],
                                         rb[:], start=(pool_i == 0),
                                         stop=(pool_i == n_pool - 1))
                        pool_i += 1
                # drain the row accumulator to SBUF then DRAM
                yrow_sb = spool.tile([1, N_MOL], F32, tag="yrow_sb")
                nc.vector.tensor_copy(yrow_sb[:], yrow_ps[:])
                nc.sync.dma_start(yrow_d[h:h + 1, :], yrow_sb[:])

            out_sb = spool.tile([B, HPC], F32, tag="out_sb")
            nc.scalar.activation(out_sb[:], pool_ps[:], AF.Copy, scale=0.001)
            nc.sync.dma_start(out_d, out_sb[:])

    nc.compile()
    return nc


_NC = None


def _get_nc():
    global _NC
    if _NC is None:
        _NC = build()
    return _NC


def make_in_maps(mol_feats, fused_feats, Wmu, bmu, mol_batch):
    """Host-side sharding: per-core input dicts."""
    bf = ml_dtypes.bfloat16
    molT = np.concatenate([np.asarray(mol_feats, np.float32).T,
                           np.ones((1, N_MOL), np.float32)], axis=0)
    molT = np.ascontiguousarray(molT).astype(bf)
    fusedT = np.ascontiguousarray(np.asarray(fused_feats, np.float32).T).astype(bf)
    Wmu = np.asarray(Wmu, np.float32)
    bmu = np.asarray(bmu, np.float32)
    mb = np.asarray(mol_batch).astype(np.int64)
    masks = np.zeros((128, A2T * B), np.float32)
    for t in range(A2T):
        seg = mb[t * 128:(t + 1) * 128]
        masks[np.arange(128), t * B + seg] = 1.0
    masks = masks.astype(bf)

    in_maps = []
    for c in range(N_CORES):
        h0 = c * HPC
        wmol = np.ascontiguousarray(
            np.concatenate([Wmu[:HID, h0:h0 + HPC], bmu[None, h0:h0 + HPC]],
                           axis=0)).astype(bf)
        wpro = np.ascontiguousarray(Wmu[HID:, h0:h0 + HPC]).astype(bf)
        in_maps.append({
            "molT": molT, "fusedT": fusedT,
            "wmol": wmol, "wpro": wpro, "masks": masks,
        })
    return in_maps


def _elu(v):
    return np.where(v > 0, v, np.expm1(v))


def combine(results, mol_batch):
    """Per-core outputs -> pooled [B, HEADS] f32 (already * 1e-3)."""
    mb = np.asarray(mol_batch).astype(np.int64)
    pooled = np.zeros((B, HEADS), np.float32)
    for c in range(N_CORES):
        h0 = c * HPC
        pooled[:, h0:h0 + HPC] += results[c]["out"]
        yrow = results[c]["yrow"]          # [HPC, N] f32
        for h in range(HPC):
            pooled[:, h0 + h] += 1e-3 * np.bincount(
                mb, weights=yrow[h].astype(np.float64), minlength=B
            ).astype(np.float32)
    return pooled


def finish(pooled, W1, b1, W2, b2):
    y = _elu(pooled @ np.asarray(W1, np.float32) + np.asarray(b1, np.float32))
    return (y @ np.asarray(W2, np.float32) + np.asarray(b2, np.float32)).astype(np.float32)


def kernel(mol_feats, fused_feats, Wmu, bmu, W1, b1, W2, b2, mol_batch,
           num_graphs, **_unused):
    nc = _get_nc()
    in_maps = make_in_maps(mol_feats, fused_feats, Wmu, bmu, mol_batch)
    res = run_bass_kernel_spmd(nc, in_maps, core_ids=list(range(N_CORES)))
    pooled = combine(res.results, mol_batch)
    return finish(pooled, W1, b1, W2, b2)
